# revision 1
# baseline (speedup 1.0000x reference)
"""Trainium2 Bass kernel for BertMoEExpertPool (forward_all_experts).

Computes, for every expert e of E=8:
    inter = gelu(hidden @ Wi[e] + bi[e])          # [N, F]
    out[e] = inter @ Wo[e] + bo[e]                # [N, H]

Sharding: expert-parallel - one expert per NeuronCore (8 cores), with
hidden_states replicated. No collectives; per-core outputs are stacked
on the host into the full [E, N, H] result.

Per-core design (all matmul operands bf16, PSUM f32):
- hidden is fed pre-transposed ([H, N]) and cast to bf16 on the host.
- matmul1 computes inter^T (F on partitions): stationary = Wi 128x128
  chunk, moving = 512-token slab of hidden -> every MM is a full
  512-row pass. bias+gelu fuse into the PSUM->SBUF evacuation on
  ScalarE with a bf16 output cast.
- matmul2 is output-transposed: stationary = Wo 128x128 chunk
  (F-contraction on partitions), moving = inter^T slab -> again full
  512-row MMs, accumulating out^T[H-chunk, 512 tokens] over the 24
  F-chunks in a single PSUM bank. bo adds during the DVE evacuation
  (per-partition scalar); the [H, N] result is transposed on the host.

Every matmul therefore streams the maximum 512 rows per stationary
load, minimizing LDWEIGHTS count (2304 total vs 3840 in the f32r
variant), and bf16 weights take the fast-weight-load path.
"""

import os
from contextlib import ExitStack

import numpy as np

import concourse.bass as bass  # noqa: F401  (bass types used via bacc/tile)
import concourse.mybir as mybir
from concourse import bacc
from concourse.bass_utils import run_bass_kernel_spmd
from concourse.tile import TileContext

E, H, F, N = 8, 768, 3072, 4096
P = 128
KH = H // P  # 6   k-tiles of matmul1 / out-chunks of matmul2
KF = F // P  # 24  f-chunks of matmul1 / k-tiles of matmul2
NB = int(os.environ.get("MOE_NB", "256"))  # token-block width (rows per MM)
NBLK = N // NB
F32 = mybir.dt.float32
BF16 = mybir.dt.bfloat16
# benchmarking only: repeat the whole computation REPS times on-device so
# per-iteration HW time can be extracted from wall-clock differences.
REPS = int(os.environ.get("MOE_REPS", "1"))
# sim-only: CoreSim has no Gelu numerics; Identity keeps layout checks valid.
_ACT_FN = (
    mybir.ActivationFunctionType.Identity
    if os.environ.get("MOE_SIM_NOGELU")
    else mybir.ActivationFunctionType.Gelu
)
# bench-only decomposition knobs (break correctness; for stall hunting)
_SKIP_MM2 = os.environ.get("MOE_SKIP_MM2") == "1"
_CONST_INTER = os.environ.get("MOE_CONST_INTER") == "1"
_SKIP_OUTDMA = os.environ.get("MOE_SKIP_OUTDMA") == "1"


def _build(reps=None):
    if reps is None:
        reps = REPS
    nc = bacc.Bacc("TRN2", target_bir_lowering=False, debug=False, num_devices=E)

    hid_d = nc.dram_tensor("hidden_t", [H, N], BF16, kind="ExternalInput")
    wi_d = nc.dram_tensor("wi", [H, F], BF16, kind="ExternalInput")
    bi_d = nc.dram_tensor("bi", [F], F32, kind="ExternalInput")
    wo_d = nc.dram_tensor("wo", [F, H], BF16, kind="ExternalInput")
    bo_d = nc.dram_tensor("bo", [H], F32, kind="ExternalInput")
    out_d = nc.dram_tensor("out_t", [H, N], F32, kind="ExternalOutput")

    hid_v = hid_d.ap().rearrange("(a p) n -> p a n", p=P)  # [128, KH, N]
    wi_v = wi_d.ap().rearrange("(a p) f -> p a f", p=P)  # [128, KH, F]
    wo_v = wo_d.ap().rearrange("(a p) h -> p a h", p=P)  # [128, KF, H]

    def _bufs(name, default):
        return int(os.environ.get(name, str(default)))

    with TileContext(nc) as tc, ExitStack() as ctx:
        wpool = ctx.enter_context(tc.tile_pool(name="wpool", bufs=1))
        hpool = ctx.enter_context(tc.tile_pool(name="hpool", bufs=_bufs("MOE_HBUFS", 2)))
        ipool = ctx.enter_context(tc.tile_pool(name="ipool", bufs=_bufs("MOE_IBUFS", 2)))
        opool = ctx.enter_context(tc.tile_pool(name="opool", bufs=_bufs("MOE_OBUFS", 3)))
        ps1p = ctx.enter_context(
            tc.tile_pool(name="ps1p", bufs=_bufs("MOE_PS1BUFS", 4), space="PSUM")
        )
        ps2p = ctx.enter_context(
            tc.tile_pool(name="ps2p", bufs=_bufs("MOE_PS2BUFS", 3), space="PSUM")
        )

        # biases (f32, per-partition layouts)
        bi_sb = wpool.tile([P, KF], F32)
        nc.sync.dma_start(out=bi_sb[:, :], in_=bi_d.ap().rearrange("(a p) -> p a", p=P))
        bo_sb = wpool.tile([P, KH], F32)
        nc.sync.dma_start(out=bo_sb[:, :], in_=bo_d.ap().rearrange("(a p) -> p a", p=P))

        # DMA order = consumption order: block 0's tokens interleaved with
        # wi's first f-chunk per a-tile (so MM(a0, fc0) can issue almost
        # immediately), then the rest of wi per f-chunk, block 1's tokens,
        # then wo per f-chunk ahead of the first matmul2.
        wi_sb = wpool.tile([P, KH, F], BF16)
        wo_sb = wpool.tile([P, KF, H], BF16)
        hid0 = wpool.tile([P, KH, NB], BF16)
        for a in range(KH):
            nc.sync.dma_start(out=hid0[:, a, :], in_=hid_v[:, a, 0:NB])
            nc.sync.dma_start(out=wi_sb[:, a, 0:P], in_=wi_v[:, a, 0:P])
        for fc in range(1, KF):
            nc.sync.dma_start(
                out=wi_sb[:, :, fc * P : (fc + 1) * P],
                in_=wi_v[:, :, fc * P : (fc + 1) * P],
            )
        hid1 = wpool.tile([P, KH, NB], BF16)
        nc.sync.dma_start(out=hid1[:, :, :], in_=hid_v[:, :, NB : 2 * NB])
        for fc in range(KF):
            nc.sync.dma_start(out=wo_sb[:, fc, :], in_=wo_v[:, fc, :])
        pre_hid = [hid0, hid1]

        rep_ctx = tc.For_i(0, reps, 1) if reps > 1 else None
        if rep_ctx is not None:
            ctx.enter_context(rep_ctx)

        # matmul1: inter^T[fc] = gelu(Wi^T @ hid + bi), one NB-row MM per
        # (fc, a-tile), accumulated over the 6 a-tiles.
        def mm1_group(fc, hid, inter):
            ps1 = ps1p.tile([P, NB], F32, tag="ps1")
            for a in range(KH):
                nc.tensor.matmul(
                    ps1[:, :],
                    wi_sb[:, a, fc * P : (fc + 1) * P],
                    hid[:, a, :],
                    start=(a == 0),
                    stop=(a == KH - 1),
                )
            nc.scalar.activation(
                inter[:, fc, :],
                ps1[:, :],
                _ACT_FN,
                bias=bi_sb[:, fc : fc + 1],
            )

        # matmul2: out^T[h-chunk] = Wo^T-chunk @ inter^T, one NB-row MM per
        # (h-chunk, fc), accumulated over the 24 f-chunks in one bank.
        def mm2_group(b, h, inter):
            ps2 = ps2p.tile([P, NB], F32, tag="ps2")
            for fc in range(KF):
                nc.tensor.matmul(
                    ps2[:, :],
                    wo_sb[:, fc, h * P : (h + 1) * P],
                    inter[:, fc, :],
                    start=(fc == 0),
                    stop=(fc == KF - 1),
                )
            osb = opool.tile([P, NB], F32, tag="osb")
            nc.vector.tensor_scalar_add(osb[:, :], ps2[:, :], bo_sb[:, h : h + 1])
            if not _SKIP_OUTDMA:
                nc.sync.dma_start(
                    out=out_d[h * P : (h + 1) * P, b * NB : (b + 1) * NB],
                    in_=osb[:, :],
                )

        # MOE_PIPE=1: interleave mm1 groups of block b with mm2 groups of
        # block b-1 so ScalarE/VectorE/DMA drain continuously. Default is the
        # plain per-block order (measured faster at NB=256).
        pipe = os.environ.get("MOE_PIPE", "0") == "1"
        G = KF // KH  # mm1 groups per mm2 group
        prev_inter = None
        for b in range(NBLK):
            if b < len(pre_hid):
                hid = pre_hid[b]
            else:
                hid = hpool.tile([P, KH, NB], BF16, tag="hid")
                nc.sync.dma_start(out=hid[:, :, :], in_=hid_v[:, :, b * NB : (b + 1) * NB])

            inter = ipool.tile([P, KF, NB], BF16, tag="inter")
            if pipe:
                for i in range(KH):
                    if not _CONST_INTER:
                        for fc in range(G * i, G * (i + 1)):
                            mm1_group(fc, hid, inter)
                    if prev_inter is not None and not _SKIP_MM2:
                        mm2_group(b - 1, i, prev_inter)
                prev_inter = inter
            else:
                if not _CONST_INTER:
                    for fc in range(KF):
                        mm1_group(fc, hid, inter)
                if not _SKIP_MM2:
                    for h in range(KH):
                        mm2_group(b, h, inter)
        if pipe and not _SKIP_MM2:
            for h in range(KH):
                mm2_group(NBLK - 1, h, prev_inter)

    nc.compile()
    return nc


_cached_nc = {}


def _get_nc(reps=None):
    if reps is None:
        reps = REPS
    if reps not in _cached_nc:
        _cached_nc[reps] = _build(reps)
    return _cached_nc[reps]


def kernel(hidden_states, Wi, bi, Wo, bo):
    import ml_dtypes

    bf16 = ml_dtypes.bfloat16
    nc = _get_nc()
    hid_t = np.ascontiguousarray(
        np.asarray(hidden_states, dtype=np.float32).T.astype(bf16)
    )
    Wi = np.asarray(Wi, dtype=np.float32)
    bi = np.asarray(bi, dtype=np.float32)
    Wo = np.asarray(Wo, dtype=np.float32)
    bo = np.asarray(bo, dtype=np.float32)
    in_maps = [
        {
            "hidden_t": hid_t,
            "wi": np.ascontiguousarray(Wi[e].astype(bf16)),
            "bi": np.ascontiguousarray(bi[e]),
            "wo": np.ascontiguousarray(Wo[e].astype(bf16)),
            "bo": np.ascontiguousarray(bo[e]),
        }
        for e in range(E)
    ]
    kwargs = {}
    if os.environ.get("MOE_TRACE"):
        kwargs = {"trace": True, "tmpdir": os.environ.get("MOE_TRACE_DIR") or None}
    res = run_bass_kernel_spmd(nc, in_maps, list(range(E)), **kwargs)
    global last_results
    last_results = res
    return np.stack(
        [np.ascontiguousarray(res.results[e]["out_t"].T) for e in range(E)], axis=0
    )


last_results = None

